# revision 1
# baseline (speedup 1.0000x reference)
"""Distributed causal self-attention kernel for Trainium2 (8 NeuronCores).

Sharding: batch x head-group grid. Core c = 2*b + g handles batch b (of 4)
and head group g (of 2, 8 heads each = 512 channels). Each core computes
Q/K/V projections for its heads over its batch, flash-style causal
attention, and a partial output projection over its 512 channels of Wp.
Host sums the two partial outputs per batch (tensor-parallel unshard).

Compute in bf16 on the PE (fp32 accumulate in PSUM), softmax in fp32.
Logits are bounded (~|2.7|) so exp needs no max-subtraction pass.

Layouts (host pre-transposes so the contraction dim lands on partitions):
  xT  [C=1024, T=2048] bf16     wqT/wkT/wvT [C=1024, 512] bf16
  wpT [512, C=1024] bf16        out [T=2048, C=1024] f32 (partial)

On-device per core:
  qT,kT = W.T-chunks @ xT-chunks  -> [512ch, 2048tok] bf16 in SBUF
  v     = xT-chunks @ wvT         -> [2048tok, 512ch], stored per key-chunk
          as v_aug [128k, head, 128] with a 64-wide ones block (even heads
          [v|1], odd heads [1|v]) so att@v_aug also accumulates the softmax
          denominator l in the opposite 64-partition half of PSUM.
  Per (head, 512-query-block): scoresT chunks [128k, 512q] = kT' @ qT
  (diagonal chunks shrunk to the causal width), exp on ACT (scale=1/8),
  bf16 triangle mask multiply on the diagonal 128x128, att@v_aug
  accumulated over key chunks in PSUM, then y/l and the Wp partial
  projection, DMA'd straight from PSUM to DRAM.
"""

import sys

if "/opt/trn_rl_repo" not in sys.path:
    sys.path.insert(0, "/opt/trn_rl_repo")

from contextlib import ExitStack

import ml_dtypes
import numpy as np

import concourse.bass as bass
import concourse.mybir as mybir
import concourse.tile as tile
from concourse import bacc
from concourse.bass_utils import run_bass_kernel_spmd
from concourse.masks import make_upper_triangular

B, T, C, H, D = 4, 2048, 1024, 16, 64
N_CORES = 8
HL = 8          # heads per core
CL = HL * D     # channels per core = 512
NCH = C // 128  # contraction chunks for projections = 8
QBS = 512       # query block size
NQB = T // QBS  # query blocks = 4
KCS = 128       # key chunk size
F32 = mybir.dt.float32
BF16 = mybir.dt.bfloat16


def build_attn(ctx: ExitStack, tc: tile.TileContext, xT, wqT, wkT, wvT, wpT, out):
    nc = tc.nc
    Exp = mybir.ActivationFunctionType.Exp

    persist = ctx.enter_context(tc.tile_pool(name="persist", bufs=1))
    psum = ctx.enter_context(tc.tile_pool(name="psum", bufs=1, space="PSUM"))
    work = ctx.enter_context(tc.tile_pool(name="work", bufs=3))

    # ---- stage inputs in SBUF ----
    xt_sb, wq_sb, wk_sb, wv_sb = [], [], [], []
    for i in range(NCH):
        for tiles, ap, label in ((wq_sb, wqT, "wq"), (wk_sb, wkT, "wk"),
                                 (wv_sb, wvT, "wv"), (xt_sb, xT, "xt")):
            width = T if label == "xt" else CL
            t = persist.tile([128, width], BF16, name=f"{label}{i}")
            nc.sync.dma_start(out=t, in_=ap[i * 128:(i + 1) * 128, :])
            tiles.append(t)

    wp_sb = []
    for i in range(CL // 128):
        t = persist.tile([128, C], BF16, name=f"wp{i}")
        nc.sync.dma_start(out=t, in_=wpT[i * 128:(i + 1) * 128, :])
        wp_sb.append(t)

    # causal triangle mask for the diagonal 128x128 block: keep k <= q
    tri32 = persist.tile([128, 128], F32, name="tri32")
    make_upper_triangular(nc, tri32, val=1.0, diag=True)
    tri = persist.tile([128, 128], BF16, name="tri")
    nc.vector.tensor_copy(out=tri, in_=tri32)

    # ---- phase A: projections ----
    # qT/kT [ch, tok]: lhsT = w chunk [128c, 128ch], rhs = xT chunk [128c, 512tok]
    qT_sb = [persist.tile([128, T], BF16, name=f"qT{i}") for i in range(CL // 128)]
    kT_sb = [persist.tile([128, T], BF16, name=f"kT{i}") for i in range(CL // 128)]
    for w_sb, dst in ((wq_sb, qT_sb), (wk_sb, kT_sb)):
        for i in range(CL // 128):
            for tt in range(T // QBS):
                pq = psum.tile([128, QBS], F32, name="pq", tag="st", bufs=3)
                for c in range(NCH):
                    nc.tensor.matmul(
                        pq,
                        lhsT=w_sb[c][:, i * 128:(i + 1) * 128],
                        rhs=xt_sb[c][:, tt * QBS:(tt + 1) * QBS],
                        start=(c == 0),
                        stop=(c == NCH - 1),
                    )
                nc.vector.tensor_copy(out=dst[i][:, tt * QBS:(tt + 1) * QBS], in_=pq)

    # v [tok, ch] stored as v_aug [128k, head, 128]; even head h: [v_h | 1],
    # odd head h: [1 | v_h] (parity picks which PSUM half holds l later).
    v_sb = [persist.tile([128, HL, 128], BF16, name=f"v{t}") for t in range(T // KCS)]
    for t in range(T // KCS):
        nc.vector.memset(v_sb[t][:, 0:HL:2, 64:128], 1.0)
        nc.vector.memset(v_sb[t][:, 1:HL:2, 0:64], 1.0)
        pv = psum.tile([128, CL], F32, name="pv", tag="st", bufs=3)
        for c in range(NCH):
            nc.tensor.matmul(
                pv,
                lhsT=xt_sb[c][:, t * KCS:(t + 1) * KCS],
                rhs=wv_sb[c],
                start=(c == 0),
                stop=(c == NCH - 1),
            )
        pv_h = pv.rearrange("p (h d) -> p h d", h=HL)
        nc.vector.tensor_copy(out=v_sb[t][:, 0:HL:2, 0:64], in_=pv_h[:, 0:HL:2, :])
        nc.vector.tensor_copy(out=v_sb[t][:, 1:HL:2, 64:128], in_=pv_h[:, 1:HL:2, :])

    # ---- phases B (attention) + C (output projection), per query block ----
    for qb in range(NQB):
        ytall = [
            work.tile([128, QBS], BF16, name=f"ytall{qb}_{cc}", tag="ytall", bufs=8)
            for cc in range(CL // 128)
        ]
        kq = QBS // KCS  # key chunks per query block = 4
        nkc = (qb + 1) * kq
        for ht in range(HL // 2):
            # heads 2*ht (rows 0:64) and 2*ht+1 (rows 64:128) interleaved: the
            # two K=64 score matmuls occupy disjoint PE row-groups and overlap
            # in the array; one exp covers both heads' score spans.
            h0, h1 = 2 * ht, 2 * ht + 1
            yt0 = psum.tile([128, QBS], F32, name="yt0", tag="yt", bufs=2)
            yt1 = psum.tile([128, QBS], F32, name="yt1", tag="yt", bufs=2)
            for kc in range(nkc):
                d = kc - qb * kq  # >= 0 on diagonal chunks
                s = d * KCS if d >= 0 else 0
                stp = psum.tile([128, 2 * QBS], F32, name="stp", tag="st", bufs=3)
                pt = work.tile([128, 2 * QBS], BF16, name="pt", tag="pt", bufs=3)
                for j, hp in ((0, 0), (1, 64)):
                    nc.tensor.matmul(
                        stp[:, j * QBS + s:(j + 1) * QBS],
                        lhsT=kT_sb[ht][hp:hp + 64, kc * KCS:(kc + 1) * KCS],
                        rhs=qT_sb[ht][hp:hp + 64, qb * QBS + s:(qb + 1) * QBS],
                        start=True,
                        stop=True,
                    )
                if s > 0:
                    # the one exp below also crosses [QBS, QBS+s) between the
                    # two heads' shrunk spans; give it defined (unread) data
                    nc.vector.memset(stp[:, QBS:QBS + s], 0.0)
                nc.scalar.activation(out=pt[:, s:2 * QBS], in_=stp[:, s:2 * QBS],
                                     func=Exp, scale=1.0 / np.sqrt(D))
                if d >= 0:
                    nc.gpsimd.tensor_mul(pt[:, s:s + KCS], pt[:, s:s + KCS], tri)
                    nc.gpsimd.tensor_mul(pt[:, QBS + s:QBS + s + KCS],
                                          pt[:, QBS + s:QBS + s + KCS], tri)
                for j, yt, h in ((0, yt0, h0), (1, yt1, h1)):
                    nc.tensor.matmul(
                        yt[:, s:QBS],
                        lhsT=v_sb[kc][:, h, :],
                        rhs=pt[:, j * QBS + s:(j + 1) * QBS],
                        start=(kc == 0),
                        stop=(kc == nkc - 1),
                    )
            # stage y and l out of PSUM (frees the yt banks fast); h0's y is in
            # rows 0:64 / l in 64:128, h1 mirrored, so lrec collects both
            # denominators full-128-aligned for one fast reciprocal (the
            # custom-DVE reciprocal mis-executes on base-partition-64 windows).
            ysb = work.tile([128, QBS], F32, name="ysb", tag="ysb", bufs=2)
            lrec = work.tile([128, QBS], F32, name="lrec", tag="lrec", bufs=2)
            nc.vector.tensor_copy(out=ysb[0:64, :], in_=yt0[0:64, :])
            nc.vector.tensor_copy(out=lrec[0:64, :], in_=yt0[64:128, :])
            nc.vector.tensor_copy(out=ysb[64:128, :], in_=yt1[64:128, :])
            nc.vector.tensor_copy(out=lrec[64:128, :], in_=yt1[0:64, :])
            rec = work.tile([128, QBS], F32, name="rec", tag="rec", bufs=2)
            nc.vector.reciprocal_approx_fast(rec, lrec)
            nc.vector.tensor_mul(ytall[ht][0:64, :], ysb[0:64, :], rec[0:64, :])
            nc.vector.tensor_mul(ytall[ht][64:128, :], ysb[64:128, :],
                                 rec[64:128, :])
        # output projection for this query block: out[tok, j] partial
        for jt in range(C // QBS):
            for tt in range(QBS // 128):
                po = psum.tile([128, QBS], F32, name="po", tag="st", bufs=3)
                for cc in range(CL // 128):
                    nc.tensor.matmul(
                        po,
                        lhsT=ytall[cc][:, tt * 128:(tt + 1) * 128],
                        rhs=wp_sb[cc][:, jt * QBS:(jt + 1) * QBS],
                        start=(cc == 0),
                        stop=(cc == CL // 128 - 1),
                    )
                ot = work.tile([128, QBS], F32, name="ot", tag="ot", bufs=3)
                nc.vector.tensor_copy(out=ot, in_=po)
                nc.sync.dma_start(
                    out=out[qb * QBS + tt * 128:qb * QBS + (tt + 1) * 128,
                            jt * QBS:(jt + 1) * QBS],
                    in_=ot,
                )


def _enable_ldw_opt():
    # the boot-time walrus flags carry --enable-ldw-opt=false, which forces a
    # serial LDWEIGHTS before every MATMUL (~107ns each); re-enable the opt
    from concourse.compiler_utils import get_compiler_flags, set_compiler_flags
    flags = [f.replace("--enable-ldw-opt=false", "--enable-ldw-opt=true")
             for f in get_compiler_flags()]
    set_compiler_flags(flags)


def build_nc():
    nc = bacc.Bacc("TRN2", target_bir_lowering=False, debug=False,
                   enable_asserts=False, num_devices=N_CORES)
    xT = nc.dram_tensor("xT", [C, T], BF16, kind="ExternalInput").ap()
    wqT = nc.dram_tensor("wqT", [C, CL], BF16, kind="ExternalInput").ap()
    wkT = nc.dram_tensor("wkT", [C, CL], BF16, kind="ExternalInput").ap()
    wvT = nc.dram_tensor("wvT", [C, CL], BF16, kind="ExternalInput").ap()
    wpT = nc.dram_tensor("wpT", [CL, C], BF16, kind="ExternalInput").ap()
    out = nc.dram_tensor("out", [T, C], F32, kind="ExternalOutput").ap()
    with tile.TileContext(nc) as tc:
        with ExitStack() as ctx:
            build_attn(ctx, tc, xT, wqT, wkT, wvT, wpT, out)
    nc.compile()
    return nc


_NC = None


def get_nc():
    global _NC
    if _NC is None:
        _NC = build_nc()
    return _NC


def make_in_maps(x, Wq, Wk, Wv, Wp):
    bf = ml_dtypes.bfloat16
    in_maps = []
    for b in range(B):
        xT_b = np.ascontiguousarray(np.asarray(x[b]).T).astype(bf)
        for g in range(2):
            sl = slice(g * CL, (g + 1) * CL)
            in_maps.append({
                "xT": xT_b,
                "wqT": np.ascontiguousarray(np.asarray(Wq)[sl, :].T).astype(bf),
                "wkT": np.ascontiguousarray(np.asarray(Wk)[sl, :].T).astype(bf),
                "wvT": np.ascontiguousarray(np.asarray(Wv)[sl, :].T).astype(bf),
                "wpT": np.ascontiguousarray(np.asarray(Wp)[:, sl].T).astype(bf),
            })
    return in_maps


def kernel(x, Wq, Wk, Wv, Wp):
    nc = get_nc()
    in_maps = make_in_maps(x, Wq, Wk, Wv, Wp)
    res = run_bass_kernel_spmd(nc, in_maps, list(range(N_CORES)))
    out = np.empty((B, T, C), dtype=np.float32)
    for b in range(B):
        out[b] = res.results[2 * b]["out"] + res.results[2 * b + 1]["out"]
    return out


if __name__ == "__main__":
    rng = np.random.default_rng(0)
    ins = {
        "x": rng.standard_normal((B, T, C), dtype=np.float32),
        "Wq": (rng.standard_normal((C, C), dtype=np.float32) * 0.02),
        "Wk": (rng.standard_normal((C, C), dtype=np.float32) * 0.02),
        "Wv": (rng.standard_normal((C, C), dtype=np.float32) * 0.02),
        "Wp": (rng.standard_normal((C, C), dtype=np.float32) * 0.02),
    }
    got = kernel(**ins)
    print("kernel output", got.shape, got.dtype)



# revision 7
# speedup vs baseline: 1.1719x; 1.1719x over previous
"""Distributed causal self-attention kernel for Trainium2 (8 NeuronCores).

Sharding: batch x head-group grid. Core c = 2*b + g handles batch b (of 4)
and head group g (of 2, 8 heads each = 512 channels). Each core computes
Q/K/V projections for its heads over its batch, flash-style causal
attention, and a partial output projection over its 512 channels of Wp.
Host sums the two partial outputs per batch (tensor-parallel unshard).

Compute in bf16 on the PE (fp32 accumulate in PSUM), softmax in fp32.
Logits are bounded (~|2.7|) so exp needs no max-subtraction pass.

Layouts (host pre-transposes so the contraction dim lands on partitions):
  xT  [C=1024, T=2048] bf16     wqT/wkT/wvT [C=1024, 512] bf16
  wpT [512, C=1024] bf16        out [T=2048, C=1024] bf16 (partial)

On-device per core:
  qT,kT = W.T-chunks @ xT-chunks  -> [512ch, 2048tok] bf16 in SBUF
          (PSUM drained by the Scalar engine, idle during projections)
  v     = xT-chunks @ wvT         -> [2048tok, 512ch], stored per key-chunk
          as v_aug [128k, head, 128] with a 64-wide ones block (even heads
          [v|1], odd heads [1|v]) so att@v_aug also accumulates the softmax
          denominator l in the opposite 64-partition half of PSUM.
  Per (head-pair, 512-query-block): scoresT chunks [128k, 2x512q] = kT' @ qT
  with the two heads in disjoint PE row-halves. Diagonal chunks shrink to
  the causal width and head1's span is left-shifted so the two spans are
  contiguous (one exp per chunk); causality is enforced by accumulating a
  -1000 strict-lower triangle onto the 128x128 diagonal square with an
  identity matmul, so exp(scale*(s-1000)) == 0 and no mask multiply is
  needed. Scores for chunk kc+1 are emitted before att@v of chunk kc so
  the PE never head-of-line blocks on the Scalar exp. The output
  projection of query block qb-1 is interleaved into the attention loop
  of qb to fill PE slack, then DMA'd to DRAM as bf16.
"""

import sys

if "/opt/trn_rl_repo" not in sys.path:
    sys.path.insert(0, "/opt/trn_rl_repo")

from contextlib import ExitStack

import ml_dtypes
import numpy as np

import concourse.bass as bass
import concourse.mybir as mybir
import concourse.tile as tile
from concourse import bacc
from concourse.bass_utils import run_bass_kernel_spmd
from concourse.masks import make_identity, make_lower_triangular

B, T, C, H, D = 4, 2048, 1024, 16, 64
N_CORES = 8
HL = 8          # heads per core
CL = HL * D     # channels per core = 512
NCH = C // 128  # contraction chunks for projections = 8
QBS = 512       # query block size
NQB = T // QBS  # query blocks = 4
KCS = 128       # key chunk size
F32 = mybir.dt.float32
BF16 = mybir.dt.bfloat16
NEG = -1000.0   # causal mask additive value (exp(scale*(s+NEG)) == 0)


def build_attn(ctx: ExitStack, tc: tile.TileContext, xT, wqT, wkT, wvT, wpT, out):
    nc = tc.nc
    Exp = mybir.ActivationFunctionType.Exp
    Copy = mybir.ActivationFunctionType.Copy

    persist = ctx.enter_context(tc.tile_pool(name="persist", bufs=1))
    psum = ctx.enter_context(tc.tile_pool(name="psum", bufs=1, space="PSUM"))
    work = ctx.enter_context(tc.tile_pool(name="work", bufs=3))

    # ---- stage inputs in SBUF; DMA in consumption order ----
    wq_sb = [persist.tile([128, CL], BF16, name=f"wq{i}") for i in range(NCH)]
    wk_sb = [persist.tile([128, CL], BF16, name=f"wk{i}") for i in range(NCH)]
    wv_sb = [persist.tile([128, CL], BF16, name=f"wv{i}") for i in range(NCH)]
    xt_sb = [persist.tile([128, T], BF16, name=f"xt{i}") for i in range(NCH)]
    wp_sb = [persist.tile([128, C], BF16, name=f"wp{i}") for i in range(CL // 128)]

    for i in range(NCH):
        nc.sync.dma_start(out=wq_sb[i], in_=wqT[i * 128:(i + 1) * 128, :])
    for i in range(NCH):  # first query-block columns of x
        nc.sync.dma_start(out=xt_sb[i][:, 0:QBS], in_=xT[i * 128:(i + 1) * 128, 0:QBS])
    for i in range(NCH):
        nc.sync.dma_start(out=wk_sb[i], in_=wkT[i * 128:(i + 1) * 128, :])
    for i in range(NCH):
        nc.sync.dma_start(out=wv_sb[i], in_=wvT[i * 128:(i + 1) * 128, :])
    for tt in range(1, NQB):
        for i in range(NCH):
            nc.sync.dma_start(out=xt_sb[i][:, tt * QBS:(tt + 1) * QBS],
                              in_=xT[i * 128:(i + 1) * 128, tt * QBS:(tt + 1) * QBS])
    for i in range(CL // 128):
        nc.sync.dma_start(out=wp_sb[i], in_=wpT[i * 128:(i + 1) * 128, :])

    # causal additive mask for the diagonal 128x128 square: NEG where k > q
    msk32 = persist.tile([128, 128], F32, name="msk32")
    make_lower_triangular(nc, msk32, val=NEG, diag=False)
    mneg = persist.tile([128, 128], BF16, name="mneg")
    nc.gpsimd.tensor_copy(out=mneg, in_=msk32)
    id32 = persist.tile([128, 128], F32, name="id32")
    make_identity(nc, id32)
    ident = persist.tile([128, 128], BF16, name="ident")
    nc.gpsimd.tensor_copy(out=ident, in_=id32)

    # ---- phase A: projections (tt-blocked to chase the x DMA) ----
    qT_sb = [persist.tile([128, T], BF16, name=f"qT{i}") for i in range(CL // 128)]
    kT_sb = [persist.tile([128, T], BF16, name=f"kT{i}") for i in range(CL // 128)]
    v_sb = [persist.tile([128, HL, 128], BF16, name=f"v{t}") for t in range(T // KCS)]

    for tt in range(NQB):
        for w_sb, dst in ((wq_sb, qT_sb), (wk_sb, kT_sb)):
            for i in range(CL // 128):
                pq = psum.tile([128, QBS], F32, name="pq", tag="st", bufs=2)
                for c in range(NCH):
                    nc.tensor.matmul(
                        pq,
                        lhsT=w_sb[c][:, i * 128:(i + 1) * 128],
                        rhs=xt_sb[c][:, tt * QBS:(tt + 1) * QBS],
                        start=(c == 0),
                        stop=(c == NCH - 1),
                    )
                # drain on the Scalar engine (idle in this phase)
                nc.scalar.activation(out=dst[i][:, tt * QBS:(tt + 1) * QBS],
                                     in_=pq, func=Copy)
        for t4 in range(QBS // KCS):
            t = tt * (QBS // KCS) + t4
            nc.gpsimd.memset(v_sb[t][:, 0:HL:2, 64:128], 1.0)
            nc.gpsimd.memset(v_sb[t][:, 1:HL:2, 0:64], 1.0)
            pv = psum.tile([128, CL], F32, name="pv", tag="st", bufs=2)
            for c in range(NCH):
                nc.tensor.matmul(
                    pv,
                    lhsT=xt_sb[c][:, t * KCS:(t + 1) * KCS],
                    rhs=wv_sb[c],
                    start=(c == 0),
                    stop=(c == NCH - 1),
                )
            pv_h = pv.rearrange("p (h d) -> p h d", h=HL)
            nc.vector.tensor_copy(out=v_sb[t][:, 0:HL:2, 0:64], in_=pv_h[:, 0:HL:2, :])
            nc.vector.tensor_copy(out=v_sb[t][:, 1:HL:2, 64:128], in_=pv_h[:, 1:HL:2, :])

    # ---- phases B (attention) + C (output projection), per query block ----
    # C(qb-1) po-groups are interleaved into B(qb)'s chunk loop.
    ytall_all = {}

    def emit_po_group(qb, g):
        jt, tt2 = g // (QBS // 128), g % (QBS // 128)
        ytall = ytall_all[qb]
        last_qb = qb == NQB - 1
        po = psum.tile([128, QBS], F32, name="po",
                       tag=("st" if last_qb else "po"),
                       bufs=(2 if last_qb else 1))
        for cc in range(CL // 128):
            nc.tensor.matmul(
                po,
                lhsT=ytall[cc][:, tt2 * 128:(tt2 + 1) * 128],
                rhs=wp_sb[cc][:, jt * QBS:(jt + 1) * QBS],
                start=(cc == 0),
                stop=(cc == CL // 128 - 1),
            )
        ot = work.tile([128, QBS], BF16, name="ot", tag="ot", bufs=3)
        nc.vector.tensor_copy(out=ot, in_=po)
        nc.sync.dma_start(
            out=out[qb * QBS + tt2 * 128:qb * QBS + (tt2 + 1) * 128,
                    jt * QBS:(jt + 1) * QBS],
            in_=ot,
        )

    for qb in range(NQB):
        ytall = [
            work.tile([128, QBS], BF16, name=f"ytall{qb}_{cc}", tag="ytall", bufs=8)
            for cc in range(CL // 128)
        ]
        ytall_all[qb] = ytall
        kq = QBS // KCS  # key chunks per query block = 4
        nkc = (qb + 1) * kq
        po_at = {nkc // 3: 0, (2 * nkc) // 3: 1} if qb > 0 else {}
        for ht in range(HL // 2):
            # heads 2*ht (rows 0:64) and 2*ht+1 (rows 64:128): the two K=64
            # score matmuls occupy disjoint PE row-groups and overlap in the
            # array; one exp covers both heads' score spans.
            h0, h1 = 2 * ht, 2 * ht + 1
            yt0 = psum.tile([128, QBS], F32, name="yt0", tag="yt", bufs=3)
            yt1 = psum.tile([128, QBS], F32, name="yt1", tag="yt", bufs=3)

            stps, pts, avs = [], [], []

            def emit_scores_exp(kc, qb=qb, ht=ht):
                d = kc - qb * kq  # >= 0 on diagonal chunks
                s = d * KCS if d >= 0 else 0
                stp = psum.tile([128, 2 * QBS], F32, name="stp", tag="st", bufs=2)
                pt = work.tile([128, 2 * QBS], BF16, name="pt", tag="pt", bufs=3)
                # head j span: j==0 -> [s, QBS); j==1 -> [QBS, 2*QBS - s)
                # (left-shifted by s so the two spans are contiguous)
                for j, hp in ((0, 0), (1, 64)):
                    o = -s if j else 0
                    nc.tensor.matmul(
                        stp[:, j * QBS + s + o:(j + 1) * QBS + o],
                        lhsT=kT_sb[ht][hp:hp + 64, kc * KCS:(kc + 1) * KCS],
                        rhs=qT_sb[ht][hp:hp + 64, qb * QBS + s:(qb + 1) * QBS],
                        start=True,
                        stop=(d < 0),
                    )
                if d >= 0:
                    # accumulate the -1000 strict-lower triangle onto the
                    # 128x128 diagonal square of each head's span
                    for j in (0, 1):
                        o = -s if j else 0
                        nc.tensor.matmul(
                            stp[:, j * QBS + s + o:j * QBS + s + o + KCS],
                            lhsT=ident,
                            rhs=mneg,
                            start=False,
                            stop=True,
                        )
                nc.scalar.activation(out=pt[:, s:2 * QBS - s],
                                     in_=stp[:, s:2 * QBS - s],
                                     func=Exp, scale=1.0 / np.sqrt(D))
                stps.append(stp)
                pts.append(pt)

            def emit_av(kc, qb=qb, ht=ht, h0=h0, h1=h1, yt0=yt0, yt1=yt1,
                        nkc=nkc):
                d = kc - qb * kq
                s = d * KCS if d >= 0 else 0
                pt = pts[kc]
                for j, yt, h in ((0, yt0, h0), (1, yt1, h1)):
                    o = -s if j else 0
                    nc.tensor.matmul(
                        yt[:, s:QBS],
                        lhsT=v_sb[kc][:, h, :],
                        rhs=pt[:, j * QBS + s + o:(j + 1) * QBS + o],
                        start=(kc == 0),
                        stop=(kc == nkc - 1),
                    )

            emit_scores_exp(0)
            for kc in range(1, nkc):
                emit_scores_exp(kc)
                emit_av(kc - 1)
                if kc in po_at:
                    emit_po_group(qb - 1, ht * 2 + po_at[kc])
            emit_av(nkc - 1)

            # normalize: h0's y is in yt0 rows 0:64 / l in 64:128, h1
            # mirrored; collect both denominators full-128-aligned for one
            # fast reciprocal (the custom-DVE reciprocal mis-executes on
            # base-partition-64 windows), then multiply straight from PSUM.
            lrec = work.tile([128, QBS], F32, name="lrec", tag="lrec", bufs=2)
            nc.vector.tensor_copy(out=lrec[0:64, :], in_=yt0[64:128, :])
            nc.vector.tensor_copy(out=lrec[64:128, :], in_=yt1[0:64, :])
            rec = work.tile([128, QBS], F32, name="rec", tag="rec", bufs=2)
            nc.vector.reciprocal_approx_fast(rec, lrec)
            nc.vector.tensor_mul(ytall[ht][0:64, :], yt0[0:64, :], rec[0:64, :])
            nc.vector.tensor_mul(ytall[ht][64:128, :], yt1[64:128, :],
                                 rec[64:128, :])

    for g in range(2 * (QBS // 128)):  # C for the last query block
        emit_po_group(NQB - 1, g)


def _enable_ldw_opt():
    # the boot-time walrus flags carry --enable-ldw-opt=false, which forces a
    # serial LDWEIGHTS before every MATMUL (~107ns each); re-enable the opt
    from concourse.compiler_utils import get_compiler_flags, set_compiler_flags
    flags = [f.replace("--enable-ldw-opt=false", "--enable-ldw-opt=true")
             for f in get_compiler_flags()]
    set_compiler_flags(flags)


def build_nc():
    _enable_ldw_opt()
    nc = bacc.Bacc("TRN2", target_bir_lowering=False, debug=False,
                   enable_asserts=False, num_devices=N_CORES)
    xT = nc.dram_tensor("xT", [C, T], BF16, kind="ExternalInput").ap()
    wqT = nc.dram_tensor("wqT", [C, CL], BF16, kind="ExternalInput").ap()
    wkT = nc.dram_tensor("wkT", [C, CL], BF16, kind="ExternalInput").ap()
    wvT = nc.dram_tensor("wvT", [C, CL], BF16, kind="ExternalInput").ap()
    wpT = nc.dram_tensor("wpT", [CL, C], BF16, kind="ExternalInput").ap()
    out = nc.dram_tensor("out", [T, C], BF16, kind="ExternalOutput").ap()
    with tile.TileContext(nc) as tc:
        with ExitStack() as ctx:
            build_attn(ctx, tc, xT, wqT, wkT, wvT, wpT, out)
    nc.compile()
    return nc


_NC = None


def get_nc():
    global _NC
    if _NC is None:
        _NC = build_nc()
    return _NC


def make_in_maps(x, Wq, Wk, Wv, Wp):
    bf = ml_dtypes.bfloat16
    in_maps = []
    for b in range(B):
        xT_b = np.ascontiguousarray(np.asarray(x[b]).T).astype(bf)
        for g in range(2):
            sl = slice(g * CL, (g + 1) * CL)
            in_maps.append({
                "xT": xT_b,
                "wqT": np.ascontiguousarray(np.asarray(Wq)[sl, :].T).astype(bf),
                "wkT": np.ascontiguousarray(np.asarray(Wk)[sl, :].T).astype(bf),
                "wvT": np.ascontiguousarray(np.asarray(Wv)[sl, :].T).astype(bf),
                "wpT": np.ascontiguousarray(np.asarray(Wp)[:, sl].T).astype(bf),
            })
    return in_maps


def kernel(x, Wq, Wk, Wv, Wp):
    nc = get_nc()
    in_maps = make_in_maps(x, Wq, Wk, Wv, Wp)
    res = run_bass_kernel_spmd(nc, in_maps, list(range(N_CORES)))
    out = np.empty((B, T, C), dtype=np.float32)
    for b in range(B):
        out[b] = (res.results[2 * b]["out"].astype(np.float32)
                  + res.results[2 * b + 1]["out"].astype(np.float32))
    return out


if __name__ == "__main__":
    rng = np.random.default_rng(0)
    ins = {
        "x": rng.standard_normal((B, T, C), dtype=np.float32),
        "Wq": (rng.standard_normal((C, C), dtype=np.float32) * 0.02),
        "Wk": (rng.standard_normal((C, C), dtype=np.float32) * 0.02),
        "Wv": (rng.standard_normal((C, C), dtype=np.float32) * 0.02),
        "Wp": (rng.standard_normal((C, C), dtype=np.float32) * 0.02),
    }
    got = kernel(**ins)
    print("kernel output", got.shape, got.dtype)
